# revision 24
# baseline (speedup 1.0000x reference)
"""ADD-S (symmetric) pose loss kernel for Trainium2, 8 NeuronCores.

Sharding: data-parallel over the batch dim B=8 -> one batch element per core.
Each core computes sum_n [ min_dist(n) * conf(n) - W*log(conf(n)) ] for its
4096 points, returned as [128,1] per-partition partial sums; the host sums the
8*128 partials and divides by B*N.

Device algorithm (per core, N = 4096 points):
  1. Elementwise prologue on DVE in a SoA layout ([128 partitions, 32 free],
     point n lives at (p, f) = (n >> 5, n & 31)):
       - quat -> rotation via the unnormalized form R = M / |q|^2
       - points_model = R_gt @ (points - t_gt)
       - points_pred  = R_pred @ points_model + trans
  2. Coordinates quantized to bf16; the PE computes NEGATED squared
     distances  -d2 = -aa - bb + 2 ab  from exact bf16 hi+lo row pairs of
     -aa/-bb, so every downstream reduction is a MAX (min d2 = -max(-d2)).
     Operands (SBUF->SBUF flatten DMAs, identity order n = p*32 + f):
       lhsT [7, 4096] bf16 = [-aa_h, -aa_l, 1, 1, 2pp~_x, 2pp~_y, 2pp~_z]
       rhs  [7, 4096] bf16 = [1, 1, -bb_h, -bb_l,  q~_x,   q~_y,   q~_z]
  3. BLOCK-MAJOR matmuls in standard 128-row mode: per n-block beta the 8
     [128,512] matmuls stream all 4096 m-columns back-to-back into a ring
     of 4 [128,1024] PSUM tiles, keeping the PE continuously busy (no
     cross-block weight thrash, full p-state ramp).
  4. Two-engine PSUM drain with a UNIFORM per-block split (min/max exists
     only on DVE; ScalarE can only move, GpSimd/DMA can touch neither
     PSUM nor max).  Every block: quarter 0 is reduced straight off PSUM
     by DVE (1 elem/cycle); quarters 1-3 are copied to fp16 SBUF by
     ScalarE and folded by DVE at 2x rate (~0.52 cycles/element) in a
     tree shared across PAIRS of consecutive blocks.  The 1:3 split is
     the LP optimum for the two engines' rates, and identical blocks keep
     the PE/ACT/DVE pipeline in a steady phase instead of oscillating.
     Tree ops are emitted a few per block ("dribbled") so the in-order
     DVE queue always serves PSUM-freeing reduces promptly.
  5. dist = sqrt(max(-negmax, 1e-12)); pixel = dist*clip(conf)
     - W*ln(clip(conf)); per-partition row sums -> [128, 1] output.
"""

import numpy as np

B = 8
N = 4096
P = 128
F = N // P          # 32 free elems per partition in SoA layout
NB = N // P         # 32 n-blocks of 128
W_RATE = 0.015
SYM_CLASS_IDS = {1}

_cache = {}


def _np_f32(x):
    return np.ascontiguousarray(np.asarray(x), dtype=np.float32)


def _emit(ctx, tc, out_ap, ins):
    import concourse.bass as bass
    from concourse import mybir

    nc = tc.nc
    f32 = mybir.dt.float32
    Alu = mybir.AluOpType
    Act = mybir.ActivationFunctionType
    X = mybir.AxisListType.X

    quat, trans, conf, pose, points = (
        ins["pred_quat"], ins["pred_trans"], ins["pred_conf"],
        ins["pose"], ins["points"],
    )

    pool = ctx.enter_context(tc.tile_pool(name="main", bufs=1))

    def t(tag, shape, dtype=f32):
        return pool.tile(shape, dtype, tag=tag, name=tag)

    dma = nc.sync.dma_start

    # ---------------- input loads ----------------
    q_t = t("q_t", [P, F * 4])       # quat rows, 4 per point
    p_t = t("p_t", [P, F * 3])       # points
    tr_t = t("tr_t", [P, F * 3])     # pred_trans
    bc = t("bc", [P, 12])            # pose scalars broadcast across partitions
    conf_b = t("conf_b", [P, NB])    # conf in output (SoA-B) order

    nc.gpsimd.dma_start(out=q_t, in_=quat.rearrange("(p f) c -> p (f c)", p=P))
    nc.gpsimd.dma_start(out=p_t, in_=points.rearrange("(p f) c -> p (f c)", p=P))
    nc.gpsimd.dma_start(out=tr_t, in_=trans.rearrange("(p f) c -> p (f c)", p=P))
    nc.gpsimd.dma_start(out=bc, in_=bass.AP(tensor=pose.tensor,
                                            offset=pose.offset,
                                            ap=[[0, P], [1, 12]]))

    q3 = q_t.rearrange("p (f c) -> p f c", c=4)
    p3 = p_t.rearrange("p (f c) -> p f c", c=3)
    tr3 = tr_t.rearrange("p (f c) -> p f c", c=3)

    vec = nc.vector
    from concourse.tile import add_dep_helper

    # DMA-wait funnel: a chain of TT ops absorbs every input-DMA semaphore
    # wait (1 per instruction) so downstream TensorScalar ops, which have
    # very few HW sync-wait slots, never carry DMA waits themselves.  All
    # early DVE consumers of DMA'd tiles are order-pinned after the funnel.
    scrf = t("scrf", [P, 1])
    vec.tensor_copy(out=scrf, in_=q_t[:, 0:1])
    for dep_t in (p_t, tr_t, bc):
        last_f = vec.tensor_tensor(out=scrf, in0=scrf, in1=dep_t[:, 0:1],
                                   op=Alu.add)

    def pin(inst):
        add_dep_helper(inst.ins, last_f.ins, sync=False,
                       reason="order after input-DMA funnel")
        return inst

    # ---------------- quaternion -> unnormalized rotation ----------------
    sq = t("sq", [P, F * 4])
    pin(vec.tensor_tensor(out=sq, in0=q_t, in1=q_t, op=Alu.mult))
    sq3 = sq.rearrange("p (f c) -> p f c", c=4)
    s2 = t("s2", [P, F])
    vec.reduce_sum(s2, sq3, axis=X)
    s2c = t("s2c", [P, F])
    vec.tensor_scalar_max(s2c, s2, 1e-16)
    rec = t("rec", [P, F])
    vec.reciprocal(rec, s2c)

    # gt transform: pm_k = sum_j Rg[k,j] * (points_j - t_j)
    # pose flat layout: Rg[k][j] = bc[:, 4k+j], t[j] = bc[:, 4j+3]
    gp = nc.gpsimd
    pc = [t(f"pc{j}", [P, F]) for j in range(3)]
    for j in range(3):
        pin(vec.tensor_scalar_sub(pc[j], p3[:, :, j],
                                  bc[:, 4 * j + 3: 4 * j + 4]))
    pm = [t(f"pm{k}", [P, F]) for k in range(3)]
    for k in range(3):
        pin(vec.tensor_scalar_mul(pm[k], pc[0], bc[:, 4 * k: 4 * k + 1]))
        vec.scalar_tensor_tensor(out=pm[k], in0=pc[1],
                                 scalar=bc[:, 4 * k + 1: 4 * k + 2],
                                 in1=pm[k], op0=Alu.mult, op1=Alu.add)
        vec.scalar_tensor_tensor(out=pm[k], in0=pc[2],
                                 scalar=bc[:, 4 * k + 2: 4 * k + 3],
                                 in1=pm[k], op0=Alu.mult, op1=Alu.add)

    qw, qx, qy, qz = (q3[:, :, 0], q3[:, :, 1], q3[:, :, 2], q3[:, :, 3])
    xx, yy, zz = (sq3[:, :, 1], sq3[:, :, 2], sq3[:, :, 3])

    def dbl_prod(tag, a, b_):
        o = t(tag, [P, F])
        pin(vec.scalar_tensor_tensor(out=o, in0=a, scalar=2.0, in1=b_,
                                     op0=Alu.mult, op1=Alu.mult))
        return o

    xy2 = dbl_prod("xy2", qx, qy)
    xz2 = dbl_prod("xz2", qx, qz)
    yz2 = dbl_prod("yz2", qy, qz)
    wx2 = dbl_prod("wx2", qw, qx)
    wy2 = dbl_prod("wy2", qw, qy)
    wz2 = dbl_prod("wz2", qw, qz)

    def tt(tag, a, b_, op):
        o = t(tag, [P, F])
        vec.tensor_tensor(out=o, in0=a, in1=b_, op=op)
        return o

    b01m = tt("b01m", xy2, wz2, Alu.subtract)   # M[0][1]
    b01p = tt("b01p", xy2, wz2, Alu.add)        # M[1][0]
    b02p = tt("b02p", xz2, wy2, Alu.add)        # M[0][2]
    b02m = tt("b02m", xz2, wy2, Alu.subtract)   # M[2][0]
    b12m = tt("b12m", yz2, wx2, Alu.subtract)   # M[1][2]
    b12p = tt("b12p", yz2, wx2, Alu.add)        # M[2][1]

    a0 = tt("a0", yy, zz, Alu.add)
    a1 = tt("a1", xx, zz, Alu.add)
    a2 = tt("a2", xx, yy, Alu.add)
    u = []
    for k, ak in enumerate((a0, a1, a2)):
        uk = t(f"u{k}", [P, F])
        vec.scalar_tensor_tensor(out=uk, in0=ak, scalar=-2.0, in1=s2,
                                 op0=Alu.mult, op1=Alu.add)
        u.append(uk)

    # M rows (unnormalized R * s2):
    rows = [(u[0], b01m, b02p), (b01p, u[1], b12m), (b02m, b12p, u[2])]
    ppc = t("ppc", [P, F * 3])      # pp components, contiguous
    scr = t("scr", [P, F])
    for i, (m0, m1, m2) in enumerate(rows):
        v = ppc[:, i * F:(i + 1) * F]
        vec.tensor_tensor(out=v, in0=m0, in1=pm[0], op=Alu.mult)
        vec.tensor_tensor(out=scr, in0=m1, in1=pm[1], op=Alu.mult)
        vec.tensor_tensor(out=v, in0=v, in1=scr, op=Alu.add)
        vec.tensor_tensor(out=scr, in0=m2, in1=pm[2], op=Alu.mult)
        vec.tensor_tensor(out=v, in0=v, in1=scr, op=Alu.add)
        # pp_i = v * rec + trans_i
        vec.tensor_tensor(out=v, in0=v, in1=rec, op=Alu.mult)
        vec.tensor_tensor(out=v, in0=v, in1=tr3[:, :, i], op=Alu.add)

    bf16 = mybir.dt.bfloat16

    # quantize predicted points to bf16; -aa computed in f32 FROM the
    # quantized coords, then split into an exact bf16 hi+lo pair.
    # The five DVE-produced lhsT source rows [naa_h, naa_l, 2pp~] live in
    # ONE contiguous tile so a single flatten DMA builds lhsT rows 0-4.
    lrows = t("lrows", [P, F * 5], bf16)
    naa_h = lrows[:, 0:F]
    naa_l = lrows[:, F:2 * F]
    # pp components sit in one contiguous [128, 3*F] tile so the bf16
    # quantize, the 2x scale, and the squares are single wide ops
    ppq3 = t("ppq3", [P, F * 3], bf16)
    vec.tensor_copy(out=ppq3, in_=ppc)
    vec.tensor_scalar_mul(lrows[:, 2 * F:5 * F], ppq3, 2.0)  # exact *2
    sq3p = t("sq3p", [P, F * 3])
    vec.tensor_tensor(out=sq3p, in0=ppq3, in1=ppq3, op=Alu.mult)
    naa = t("naa", [P, F])
    vec.tensor_tensor(out=naa, in0=sq3p[:, 0:F], in1=sq3p[:, F:2 * F],
                      op=Alu.add)
    vec.tensor_tensor(out=naa, in0=naa, in1=sq3p[:, 2 * F:3 * F],
                      op=Alu.add)
    vec.tensor_scalar_mul(naa, naa, -1.0)
    vec.tensor_copy(out=naa_h, in_=naa)
    vec.tensor_tensor(out=naa_l, in0=naa, in1=naa_h, op=Alu.subtract)

    # quantized target coords + bb = |q~|^2 as exact bf16 hi+lo pair
    # (GpSimd).  bb stays POSITIVE; the sign flip for -bb comes from the
    # lhsT rows 5,6 being -1 instead.  The three coord rows and the two
    # bb rows live in contiguous tiles for batched flatten DMAs.
    gscr = t("gscr", [P, F])
    prows = t("prows", [P, F * 3], bf16)
    pcomp = [prows[:, j * F:(j + 1) * F] for j in range(3)]
    for j in range(3):
        gp.tensor_copy(out=pcomp[j], in_=p3[:, :, j])
    bb = t("bb", [P, F])
    gp.tensor_tensor(out=bb, in0=pcomp[0], in1=pcomp[0], op=Alu.mult)
    gp.tensor_tensor(out=gscr, in0=pcomp[1], in1=pcomp[1], op=Alu.mult)
    gp.tensor_tensor(out=bb, in0=bb, in1=gscr, op=Alu.add)
    gp.tensor_tensor(out=gscr, in0=pcomp[2], in1=pcomp[2], op=Alu.mult)
    gp.tensor_tensor(out=bb, in0=bb, in1=gscr, op=Alu.add)
    bbrows = t("bbrows", [P, F * 2], bf16)
    bb_h = bbrows[:, 0:F]
    bb_l = bbrows[:, F:2 * F]
    gp.tensor_copy(out=bb_h, in_=bb)
    gp.tensor_tensor(out=bb_l, in0=bb, in1=bb_h, op=Alu.subtract)

    # conf gather in block order: conf_b[p, beta] = conf[beta*128 + p]
    gp.dma_start(out=conf_b, in_=bass.AP(tensor=conf.tensor,
                                         offset=conf.offset,
                                         ap=[[1, P], [P, NB]]))

    # ---------------- conf term (early: ACT Ln table load overlaps) -------
    cc = t("cc", [P, NB])
    pin(vec.tensor_scalar_max(cc, conf_b, 1e-4))
    vec.tensor_scalar_min(cc, cc, 1.0)
    lnc = t("lnc", [P, NB])
    ln_inst = nc.scalar.activation(lnc, cc, Act.Ln)
    # prefetch the sqrt table set during the main loop (after all Ln uses;
    # the main-loop ACT copies need no table)
    sq_pre = t("sq_pre", [P, 1])
    sq_inst = nc.scalar.sqrt(sq_pre, cc[:, 0:1])
    add_dep_helper(sq_inst.ins, ln_inst.ins, sync=False,
                   reason="load sqrt ACT table after ln")

    # ---------------- matmul operands ----------------
    # Standard 128-row mode (no row tiling): logical rows 0..6 only.
    # row pairing: lhsT = [naa_h, naa_l, 2ppx, 2ppy, 2ppz, -1, -1]
    #              rhs  = [  1,     1,   qx,   qy,   qz, bb_h, bb_l]
    K_DIM = 7
    lhsT = t("lhsT", [P, N], bf16)
    rhs = t("rhs", [P, N], bf16)
    ones_t = t("ones_t", [P, F], bf16)
    vec.memset(ones_t, 1.0)
    mones_t = t("mones_t", [P, F], bf16)
    vec.memset(mones_t, -1.0)
    # operand rows built via flatten DMAs spread over three DGE queues
    # (sync / scalar / gpsimd) so the builds overlap instead of queueing.
    dma(out=lhsT[5:6, :], in_=mones_t)       # -1 rows flip bb's sign
    nc.scalar.dma_start(out=lhsT[6:7, :], in_=mones_t)
    dma(out=rhs[0:1, :], in_=ones_t)
    nc.scalar.dma_start(out=rhs[1:2, :], in_=ones_t)
    for r in range(5):
        eng = (dma, nc.scalar.dma_start, dma, nc.scalar.dma_start, dma)[r]
        eng(out=lhsT[r:r + 1, :], in_=lrows[:, r * F:(r + 1) * F])
    for i, r in enumerate(range(2, 5)):
        eng = (gp.dma_start, nc.scalar.dma_start, dma)[i]
        eng(out=rhs[r:r + 1, :], in_=prows[:, i * F:(i + 1) * F])
    gp.dma_start(out=rhs[5:6, :], in_=bbrows[:, 0:F])
    gp.dma_start(out=rhs[6:7, :], in_=bbrows[:, F:2 * F])

    # ---------------- main loop: block-major -d2 matmuls, 2-engine drain --
    CH = 1024                       # psum tile free size (2 banks)
    fp16 = mybir.dt.float16
    dirb = t("dirb", [P, NB])       # per-block max over the direct quarter
    treeb = t("treeb", [P, NB])     # per-block max over the copied quarters
    pp_psum = ctx.enter_context(tc.tile_pool(name="d2p", bufs=4, space="PSUM"))
    jpool = ctx.enter_context(tc.tile_pool(name="junk", bufs=3))
    tpool = ctx.enter_context(tc.tile_pool(name="tree", bufs=1))

    def make_tree_ops(b0, b1, jp):
        # fold the 2x3 copied quarters of blocks (b0, b1): [128,2,3072]
        # -> treeb cols b0, b1.  Returns closures to dribble into the DVE
        # stream a few per block.
        v = jp.rearrange("p (b m) -> p b m", b=2)
        tX = tpool.tile([P, 2 * 1024], fp16, tag="tX", name=f"tX{b1}")
        tY = tpool.tile([P, 2 * 1024], fp16, tag="tY", name=f"tY{b1}")
        tC = tpool.tile([P, 2 * 512], fp16, tag="tC", name=f"tC{b1}")
        tD = tpool.tile([P, 2 * 256], fp16, tag="tD", name=f"tD{b1}")
        tE = tpool.tile([P, 2 * 128], fp16, tag="tE", name=f"tE{b1}")
        tXv = tX.rearrange("p (b m) -> p b m", b=2)
        tYv = tY.rearrange("p (b m) -> p b m", b=2)
        tCv = tC.rearrange("p (b m) -> p b m", b=2)
        tDv = tD.rearrange("p (b m) -> p b m", b=2)
        tEv = tE.rearrange("p (b m) -> p b m", b=2)
        return [
            lambda: vec.tensor_tensor(out=tXv, in0=v[:, :, 0:1024],
                                      in1=v[:, :, 1024:2048], op=Alu.max),
            lambda: vec.tensor_tensor(out=tYv, in0=tXv,
                                      in1=v[:, :, 2048:3072], op=Alu.max),
            lambda: vec.tensor_tensor(out=tCv, in0=tYv[:, :, 0:512],
                                      in1=tYv[:, :, 512:1024], op=Alu.max),
            lambda: vec.tensor_tensor(out=tDv, in0=tCv[:, :, 0:256],
                                      in1=tCv[:, :, 256:512], op=Alu.max),
            lambda: vec.tensor_tensor(out=tEv, in0=tDv[:, :, 0:128],
                                      in1=tDv[:, :, 128:256], op=Alu.max),
            lambda: vec.tensor_reduce(treeb[:, b0:b0 + 1], tEv[:, 0, :],
                                      axis=X, op=Alu.max),
            lambda: vec.tensor_reduce(treeb[:, b1:b1 + 1], tEv[:, 1, :],
                                      axis=X, op=Alu.max),
        ]

    # per-chunk tail state: as soon as the trees for an 8-column chunk of
    # blocks have been emitted, fold that chunk through negate/clamp/sqrt/
    # pixel-loss so only the last chunk remains after the final block.
    md = t("md", [P, NB])
    dist = t("dist", [P, NB])
    pix = t("pix", [P, NB])
    tail_done = [0]     # columns fully pushed through the chunk tail
    cols_ready = [0]    # columns whose tree ops are fully emitted

    def emit_chunk_tails():
        while tail_done[0] + 8 <= cols_ready[0]:
            c0, c1 = tail_done[0], tail_done[0] + 8
            tail_done[0] = c1
            vec.tensor_tensor(out=md[:, c0:c1], in0=dirb[:, c0:c1],
                              in1=treeb[:, c0:c1], op=Alu.max)
            vec.tensor_scalar_mul(md[:, c0:c1], md[:, c0:c1], -1.0)
            vec.tensor_scalar_max(md[:, c0:c1], md[:, c0:c1], 1e-12)
            nc.scalar.sqrt(dist[:, c0:c1], md[:, c0:c1])
            vec.tensor_tensor(out=pix[:, c0:c1], in0=dist[:, c0:c1],
                              in1=cc[:, c0:c1], op=Alu.mult)
            vec.scalar_tensor_tensor(out=pix[:, c0:c1], in0=lnc[:, c0:c1],
                                     scalar=-W_RATE, in1=pix[:, c0:c1],
                                     op0=Alu.mult, op1=Alu.add)

    tree_q = []
    jpair = None
    for beta in range(NB):
        tiles = [pp_psum.tile([P, CH], f32, tag="ps", name=f"ps{beta}_{q}")
                 for q in range(4)]
        for q in range(4):
            for j in range(2):
                nc.tensor.matmul(
                    tiles[q][:, j * 512:(j + 1) * 512],
                    lhsT[0:K_DIM, beta * P:(beta + 1) * P],
                    rhs[0:K_DIM,
                        q * CH + j * 512: q * CH + (j + 1) * 512],
                    start=True, stop=True,
                )
        # DVE reduces quarter 0 straight off PSUM (frees the tile early)
        vec.tensor_reduce(dirb[:, beta:beta + 1], tiles[0], axis=X,
                          op=Alu.max)
        # ScalarE evacuates quarters 1-3 as fp16 into the pair buffer
        half = beta % 2
        if half == 0:
            jpair = jpool.tile([P, 2 * 3072], fp16, tag="jp",
                               name=f"jp{beta}")
        for q in (1, 2, 3):
            nc.scalar.copy(
                out=jpair[:, half * 3072 + (q - 1) * CH:
                          half * 3072 + q * CH],
                in_=tiles[q])
        # dribble pending fold-tree ops behind this block's PSUM work
        # (flush everything near the end so the tail chain starts early)
        for _ in range(5 if beta < 28 else 16):
            if tree_q:
                tree_q.pop(0)()
                if not tree_q:
                    cols_ready[0] += 2
        emit_chunk_tails()
        if half == 1:
            tree_q += make_tree_ops(beta - 1, beta, jpair)
    while tree_q:
        tree_q.pop(0)()
        if not tree_q:
            cols_ready[0] += 2
    emit_chunk_tails()
    assert tail_done[0] == NB, tail_done

    # ---------------- tail: final row sums ----------
    sums = t("sums", [P, 1])
    vec.reduce_sum(sums, pix, axis=X)
    dma(out=out_ap, in_=sums)


def _build():
    from contextlib import ExitStack

    import concourse.bacc as bacc
    import concourse.tile as tile
    from concourse import mybir

    f32 = mybir.dt.float32
    nc = bacc.Bacc("TRN2", debug=False, enable_asserts=False, num_devices=B)
    ins = {
        "pred_quat": nc.dram_tensor("pred_quat", [N, 4], f32,
                                    kind="ExternalInput").ap(),
        "pred_trans": nc.dram_tensor("pred_trans", [N, 3], f32,
                                     kind="ExternalInput").ap(),
        "pred_conf": nc.dram_tensor("pred_conf", [N, 1], f32,
                                    kind="ExternalInput").ap(),
        "pose": nc.dram_tensor("pose", [3, 4], f32, kind="ExternalInput").ap(),
        "points": nc.dram_tensor("points", [N, 3], f32,
                                 kind="ExternalInput").ap(),
    }
    out_ap = nc.dram_tensor("out_sums", [P, 1], f32, kind="ExternalOutput").ap()
    with tile.TileContext(nc) as tc:
        with ExitStack() as ctx:
            _emit(ctx, tc, out_ap, ins)
    nc.compile()
    return nc


def _get_nc():
    if "nc" not in _cache:
        _cache["nc"] = _build()
    return _cache["nc"]


def _numpy_reference(pred_quat, pred_trans, pred_conf, pose, points, cls_id):
    """Full-precision numpy fallback (used only for the non-symmetric branch)."""
    q = pred_quat.astype(np.float64)
    q = q / np.clip(np.linalg.norm(q, axis=-1, keepdims=True), 1e-8, None)
    w, x, y, z = q[..., 0], q[..., 1], q[..., 2], q[..., 3]
    r = np.stack([
        1 - 2 * (y * y + z * z), 2 * (x * y - w * z), 2 * (x * z + w * y),
        2 * (x * y + w * z), 1 - 2 * (x * x + z * z), 2 * (y * z - w * x),
        2 * (x * z - w * y), 2 * (y * z + w * x), 1 - 2 * (x * x + y * y),
    ], axis=-1).reshape(q.shape[:-1] + (3, 3))
    gt_r = pose[:, :3, :3].astype(np.float64)
    gt_t = pose[:, :3, 3].astype(np.float64)
    pc = points.astype(np.float64) - gt_t[:, None, :]
    pm = np.einsum("bkj,bnj->bnk", gt_r, pc)
    ppred = np.einsum("bnij,bnj->bni", r, pm) + pred_trans.astype(np.float64)
    tgt = points.astype(np.float64)
    if int(cls_id[0]) in SYM_CLASS_IDS:
        aa = np.sum(ppred * ppred, axis=-1)
        bb2 = np.sum(tgt * tgt, axis=-1)
        ab = np.einsum("bnd,bmd->bnm", ppred, tgt)
        d2 = aa[:, :, None] + bb2[:, None, :] - 2.0 * ab
        loss_dist = np.sqrt(np.maximum(d2, 1e-12)).min(axis=2)
    else:
        loss_dist = np.linalg.norm(ppred - tgt, axis=2)
    c = np.clip(pred_conf[..., 0].astype(np.float64), 1e-4, 1.0)
    return np.float32(np.mean(loss_dist * c - W_RATE * np.log(c)))


def kernel(pred_quat, pred_trans, pred_conf, pose, points, cls_id):
    pred_quat = _np_f32(pred_quat)
    pred_trans = _np_f32(pred_trans)
    pred_conf = _np_f32(pred_conf)
    pose = _np_f32(pose)
    points = _np_f32(points)
    cls_id = np.asarray(cls_id)

    assert pred_quat.shape == (B, N, 4), pred_quat.shape

    if int(cls_id[0]) not in SYM_CLASS_IDS:
        return np.array(
            _numpy_reference(pred_quat, pred_trans, pred_conf, pose, points,
                             cls_id),
            dtype=np.float32)

    from concourse.bass_utils import run_bass_kernel_spmd

    nc = _get_nc()
    in_maps = [
        {
            "pred_quat": np.ascontiguousarray(pred_quat[c]),
            "pred_trans": np.ascontiguousarray(pred_trans[c]),
            "pred_conf": np.ascontiguousarray(pred_conf[c]),
            "pose": np.ascontiguousarray(pose[c]),
            "points": np.ascontiguousarray(points[c]),
        }
        for c in range(B)
    ]
    res = run_bass_kernel_spmd(nc, in_maps, core_ids=list(range(B)))
    total = np.float64(0.0)
    for r in res.results:
        total += np.sum(r["out_sums"].astype(np.float64))
    return np.array(total / (B * N), dtype=np.float32)


# revision 27
# speedup vs baseline: 1.0138x; 1.0138x over previous
"""ADD-S (symmetric) pose loss kernel for Trainium2, 8 NeuronCores.

Sharding: data-parallel over the batch dim B=8 -> one batch element per core.
Each core computes sum_n [ min_dist(n) * conf(n) - W*log(conf(n)) ] for its
4096 points, returned as [128,1] per-partition partial sums; the host sums the
8*128 partials and divides by B*N.

Device algorithm (per core, N = 4096 points):
  1. Elementwise prologue on DVE in a SoA layout ([128 partitions, 32 free],
     point n lives at (p, f) = (n >> 5, n & 31)):
       - quat -> rotation via the unnormalized form R = M / |q|^2
       - points_model = R_gt @ (points - t_gt)
       - points_pred  = R_pred @ points_model + trans
  2. Coordinates quantized to bf16; the PE computes NEGATED squared
     distances  -d2 = -aa - bb + 2 ab  from exact bf16 hi+lo row pairs of
     -aa/-bb, so every downstream reduction is a MAX (min d2 = -max(-d2)).
     Operands (SBUF->SBUF flatten DMAs, identity order n = p*32 + f):
       lhsT [7, 4096] bf16 = [-aa_h, -aa_l, 1, 1, 2pp~_x, 2pp~_y, 2pp~_z]
       rhs  [7, 4096] bf16 = [1, 1, -bb_h, -bb_l,  q~_x,   q~_y,   q~_z]
  3. BLOCK-MAJOR matmuls in standard 128-row mode: per n-block beta the 8
     [128,512] matmuls stream all 4096 m-columns back-to-back into a ring
     of 4 [128,1024] PSUM tiles, keeping the PE continuously busy (no
     cross-block weight thrash, full p-state ramp).
  4. Two-engine PSUM drain with a UNIFORM per-block split (min/max exists
     only on DVE; ScalarE can only move, GpSimd/DMA can touch neither
     PSUM nor max).  Every block: quarter 0 is reduced straight off PSUM
     by DVE (1 elem/cycle); quarters 1-3 are copied to fp16 SBUF by
     ScalarE and folded by DVE at 2x rate (~0.52 cycles/element) in a
     tree shared across PAIRS of consecutive blocks.  The 1:3 split is
     the LP optimum for the two engines' rates, and identical blocks keep
     the PE/ACT/DVE pipeline in a steady phase instead of oscillating.
     Tree ops are emitted a few per block ("dribbled") so the in-order
     DVE queue always serves PSUM-freeing reduces promptly.
  5. dist = sqrt(max(-negmax, 1e-12)); pixel = dist*clip(conf)
     - W*ln(clip(conf)); per-partition row sums -> [128, 1] output.
"""

import numpy as np

B = 8
N = 4096
P = 128
F = N // P          # 32 free elems per partition in SoA layout
NB = N // P         # 32 n-blocks of 128
W_RATE = 0.015
SYM_CLASS_IDS = {1}

_cache = {}


def _np_f32(x):
    return np.ascontiguousarray(np.asarray(x), dtype=np.float32)


def _emit(ctx, tc, out_ap, ins):
    import concourse.bass as bass
    from concourse import mybir

    nc = tc.nc
    f32 = mybir.dt.float32
    Alu = mybir.AluOpType
    Act = mybir.ActivationFunctionType
    X = mybir.AxisListType.X

    quat, trans, conf, pose, points = (
        ins["pred_quat"], ins["pred_trans"], ins["pred_conf"],
        ins["pose"], ins["points"],
    )

    pool = ctx.enter_context(tc.tile_pool(name="main", bufs=1))

    def t(tag, shape, dtype=f32):
        return pool.tile(shape, dtype, tag=tag, name=tag)

    dma = nc.sync.dma_start

    # ---------------- input loads ----------------
    q_t = t("q_t", [P, F * 4])       # quat rows, 4 per point
    p_t = t("p_t", [P, F * 3])       # points
    tr_t = t("tr_t", [P, F * 3])     # pred_trans
    bc = t("bc", [P, 12])            # pose scalars broadcast across partitions
    conf_b = t("conf_b", [P, NB])    # conf in output (SoA-B) order

    # input loads spread over three DGE queues so they land in parallel
    nc.sync.dma_start(out=q_t, in_=quat.rearrange("(p f) c -> p (f c)", p=P))
    nc.scalar.dma_start(out=p_t,
                        in_=points.rearrange("(p f) c -> p (f c)", p=P))
    nc.gpsimd.dma_start(out=tr_t, in_=trans.rearrange("(p f) c -> p (f c)", p=P))
    nc.gpsimd.dma_start(out=bc, in_=bass.AP(tensor=pose.tensor,
                                            offset=pose.offset,
                                            ap=[[0, P], [1, 12]]))

    q3 = q_t.rearrange("p (f c) -> p f c", c=4)
    p3 = p_t.rearrange("p (f c) -> p f c", c=3)
    tr3 = tr_t.rearrange("p (f c) -> p f c", c=3)

    vec = nc.vector
    from concourse.tile import add_dep_helper

    # sq starts as soon as q_t lands (TT carries its own DMA wait); it is
    # emitted BEFORE the funnel so the in-order DVE queue does not stall
    # on the later input DMAs first.
    sq = t("sq", [P, F * 4])
    vec.tensor_tensor(out=sq, in0=q_t, in1=q_t, op=Alu.mult)
    sq3 = sq.rearrange("p (f c) -> p f c", c=4)

    # DMA-wait funnel: a chain of TT ops absorbs every input-DMA semaphore
    # wait (1 per instruction) so downstream TensorScalar ops, which have
    # very few HW sync-wait slots, never carry DMA waits themselves.  All
    # early DVE consumers of DMA'd tiles are order-pinned after the funnel.
    scrf = t("scrf", [P, 1])
    vec.tensor_copy(out=scrf, in_=q_t[:, 0:1])
    for dep_t in (p_t, tr_t, bc):
        last_f = vec.tensor_tensor(out=scrf, in0=scrf, in1=dep_t[:, 0:1],
                                   op=Alu.add)

    def pin(inst):
        add_dep_helper(inst.ins, last_f.ins, sync=False,
                       reason="order after input-DMA funnel")
        return inst

    # ---------------- quaternion -> unnormalized rotation ----------------
    s2 = t("s2", [P, F])
    vec.reduce_sum(s2, sq3, axis=X)
    s2c = t("s2c", [P, F])
    vec.tensor_scalar_max(s2c, s2, 1e-16)
    rec = t("rec", [P, F])
    vec.reciprocal(rec, s2c)

    # gt transform: pm_k = sum_j Rg[k,j] * (points_j - t_j)
    # pose flat layout: Rg[k][j] = bc[:, 4k+j], t[j] = bc[:, 4j+3]
    gp = nc.gpsimd
    pc = [t(f"pc{j}", [P, F]) for j in range(3)]
    for j in range(3):
        pin(vec.tensor_scalar_sub(pc[j], p3[:, :, j],
                                  bc[:, 4 * j + 3: 4 * j + 4]))
    pm = [t(f"pm{k}", [P, F]) for k in range(3)]
    for k in range(3):
        pin(vec.tensor_scalar_mul(pm[k], pc[0], bc[:, 4 * k: 4 * k + 1]))
        vec.scalar_tensor_tensor(out=pm[k], in0=pc[1],
                                 scalar=bc[:, 4 * k + 1: 4 * k + 2],
                                 in1=pm[k], op0=Alu.mult, op1=Alu.add)
        vec.scalar_tensor_tensor(out=pm[k], in0=pc[2],
                                 scalar=bc[:, 4 * k + 2: 4 * k + 3],
                                 in1=pm[k], op0=Alu.mult, op1=Alu.add)

    qw, qx, qy, qz = (q3[:, :, 0], q3[:, :, 1], q3[:, :, 2], q3[:, :, 3])
    xx, yy, zz = (sq3[:, :, 1], sq3[:, :, 2], sq3[:, :, 3])

    def dbl_prod(tag, a, b_):
        o = t(tag, [P, F])
        pin(vec.scalar_tensor_tensor(out=o, in0=a, scalar=2.0, in1=b_,
                                     op0=Alu.mult, op1=Alu.mult))
        return o

    xy2 = dbl_prod("xy2", qx, qy)
    xz2 = dbl_prod("xz2", qx, qz)
    yz2 = dbl_prod("yz2", qy, qz)
    wx2 = dbl_prod("wx2", qw, qx)
    wy2 = dbl_prod("wy2", qw, qy)
    wz2 = dbl_prod("wz2", qw, qz)

    def tt(tag, a, b_, op):
        o = t(tag, [P, F])
        vec.tensor_tensor(out=o, in0=a, in1=b_, op=op)
        return o

    b01m = tt("b01m", xy2, wz2, Alu.subtract)   # M[0][1]
    b01p = tt("b01p", xy2, wz2, Alu.add)        # M[1][0]
    b02p = tt("b02p", xz2, wy2, Alu.add)        # M[0][2]
    b02m = tt("b02m", xz2, wy2, Alu.subtract)   # M[2][0]
    b12m = tt("b12m", yz2, wx2, Alu.subtract)   # M[1][2]
    b12p = tt("b12p", yz2, wx2, Alu.add)        # M[2][1]

    a0 = tt("a0", yy, zz, Alu.add)
    a1 = tt("a1", xx, zz, Alu.add)
    a2 = tt("a2", xx, yy, Alu.add)
    u = []
    for k, ak in enumerate((a0, a1, a2)):
        uk = t(f"u{k}", [P, F])
        vec.scalar_tensor_tensor(out=uk, in0=ak, scalar=-2.0, in1=s2,
                                 op0=Alu.mult, op1=Alu.add)
        u.append(uk)

    # M rows (unnormalized R * s2):
    rows = [(u[0], b01m, b02p), (b01p, u[1], b12m), (b02m, b12p, u[2])]
    ppc = t("ppc", [P, F * 3])      # pp components, contiguous
    scr = t("scr", [P, F])
    for i, (m0, m1, m2) in enumerate(rows):
        v = ppc[:, i * F:(i + 1) * F]
        vec.tensor_tensor(out=v, in0=m0, in1=pm[0], op=Alu.mult)
        vec.tensor_tensor(out=scr, in0=m1, in1=pm[1], op=Alu.mult)
        vec.tensor_tensor(out=v, in0=v, in1=scr, op=Alu.add)
        vec.tensor_tensor(out=scr, in0=m2, in1=pm[2], op=Alu.mult)
        vec.tensor_tensor(out=v, in0=v, in1=scr, op=Alu.add)
        # pp_i = v * rec + trans_i
        vec.tensor_tensor(out=v, in0=v, in1=rec, op=Alu.mult)
        vec.tensor_tensor(out=v, in0=v, in1=tr3[:, :, i], op=Alu.add)

    bf16 = mybir.dt.bfloat16

    # quantize predicted points to bf16; -aa computed in f32 FROM the
    # quantized coords, then split into an exact bf16 hi+lo pair.
    # The five DVE-produced lhsT source rows [naa_h, naa_l, 2pp~] live in
    # ONE contiguous tile so a single flatten DMA builds lhsT rows 0-4.
    lrows = t("lrows", [P, F * 5], bf16)
    naa_h = lrows[:, 0:F]
    naa_l = lrows[:, F:2 * F]
    # pp components sit in one contiguous [128, 3*F] tile so the bf16
    # quantize, the 2x scale, and the squares are single wide ops
    ppq3 = t("ppq3", [P, F * 3], bf16)
    vec.tensor_copy(out=ppq3, in_=ppc)
    vec.tensor_scalar_mul(lrows[:, 2 * F:5 * F], ppq3, 2.0)  # exact *2
    sq3p = t("sq3p", [P, F * 3])
    vec.tensor_tensor(out=sq3p, in0=ppq3, in1=ppq3, op=Alu.mult)
    naa = t("naa", [P, F])
    vec.tensor_tensor(out=naa, in0=sq3p[:, 0:F], in1=sq3p[:, F:2 * F],
                      op=Alu.add)
    vec.tensor_tensor(out=naa, in0=naa, in1=sq3p[:, 2 * F:3 * F],
                      op=Alu.add)
    vec.tensor_scalar_mul(naa, naa, -1.0)
    vec.tensor_copy(out=naa_h, in_=naa)
    vec.tensor_tensor(out=naa_l, in0=naa, in1=naa_h, op=Alu.subtract)

    # quantized target coords + bb = |q~|^2 as exact bf16 hi+lo pair
    # (GpSimd).  bb stays POSITIVE; the sign flip for -bb comes from the
    # lhsT rows 5,6 being -1 instead.  The three coord rows and the two
    # bb rows live in contiguous tiles for batched flatten DMAs.
    gscr = t("gscr", [P, F])
    prows = t("prows", [P, F * 3], bf16)
    pcomp = [prows[:, j * F:(j + 1) * F] for j in range(3)]
    for j in range(3):
        gp.tensor_copy(out=pcomp[j], in_=p3[:, :, j])
    bb = t("bb", [P, F])
    gp.tensor_tensor(out=bb, in0=pcomp[0], in1=pcomp[0], op=Alu.mult)
    gp.tensor_tensor(out=gscr, in0=pcomp[1], in1=pcomp[1], op=Alu.mult)
    gp.tensor_tensor(out=bb, in0=bb, in1=gscr, op=Alu.add)
    gp.tensor_tensor(out=gscr, in0=pcomp[2], in1=pcomp[2], op=Alu.mult)
    gp.tensor_tensor(out=bb, in0=bb, in1=gscr, op=Alu.add)
    bbrows = t("bbrows", [P, F * 2], bf16)
    bb_h = bbrows[:, 0:F]
    bb_l = bbrows[:, F:2 * F]
    gp.tensor_copy(out=bb_h, in_=bb)
    gp.tensor_tensor(out=bb_l, in0=bb, in1=bb_h, op=Alu.subtract)

    # conf gather in block order: conf_b[p, beta] = conf[beta*128 + p]
    gp.dma_start(out=conf_b, in_=bass.AP(tensor=conf.tensor,
                                         offset=conf.offset,
                                         ap=[[1, P], [P, NB]]))

    # ---------------- conf term (early: ACT Ln table load overlaps) -------
    cc = t("cc", [P, NB])
    pin(vec.tensor_scalar_max(cc, conf_b, 1e-4))
    vec.tensor_scalar_min(cc, cc, 1.0)
    lnc = t("lnc", [P, NB])
    ln_inst = nc.scalar.activation(lnc, cc, Act.Ln)
    # prefetch the sqrt table set during the main loop (after all Ln uses;
    # the main-loop ACT copies need no table)
    sq_pre = t("sq_pre", [P, 1])
    sq_inst = nc.scalar.sqrt(sq_pre, cc[:, 0:1])
    add_dep_helper(sq_inst.ins, ln_inst.ins, sync=False,
                   reason="load sqrt ACT table after ln")

    # ---------------- matmul operands ----------------
    # Standard 128-row mode (no row tiling): logical rows 0..6 only.
    # row pairing: lhsT = [naa_h, naa_l, 2ppx, 2ppy, 2ppz, -1, -1]
    #              rhs  = [  1,     1,   qx,   qy,   qz, bb_h, bb_l]
    K_DIM = 7
    lhsT = t("lhsT", [P, N], bf16)
    rhs = t("rhs", [P, N], bf16)
    ones_t = t("ones_t", [P, F], bf16)
    vec.memset(ones_t, 1.0)
    mones_t = t("mones_t", [P, F], bf16)
    vec.memset(mones_t, -1.0)
    # operand rows built via flatten DMAs spread over three DGE queues
    # (sync / scalar / gpsimd) so the builds overlap instead of queueing.
    dma(out=lhsT[5:6, :], in_=mones_t)       # -1 rows flip bb's sign
    nc.scalar.dma_start(out=lhsT[6:7, :], in_=mones_t)
    dma(out=rhs[0:1, :], in_=ones_t)
    nc.scalar.dma_start(out=rhs[1:2, :], in_=ones_t)
    for r in range(5):
        eng = (dma, nc.scalar.dma_start, dma, nc.scalar.dma_start, dma)[r]
        eng(out=lhsT[r:r + 1, :], in_=lrows[:, r * F:(r + 1) * F])
    for i, r in enumerate(range(2, 5)):
        eng = (gp.dma_start, nc.scalar.dma_start, dma)[i]
        eng(out=rhs[r:r + 1, :], in_=prows[:, i * F:(i + 1) * F])
    gp.dma_start(out=rhs[5:6, :], in_=bbrows[:, 0:F])
    gp.dma_start(out=rhs[6:7, :], in_=bbrows[:, F:2 * F])

    # ---------------- main loop: block-major -d2 matmuls, 2-engine drain --
    CH = 1024                       # psum tile free size (2 banks)
    fp16 = mybir.dt.float16
    dirb = t("dirb", [P, NB])       # per-block max over the direct quarter
    treeb = t("treeb", [P, NB])     # per-block max over the copied quarters
    pp_psum = ctx.enter_context(tc.tile_pool(name="d2p", bufs=4, space="PSUM"))
    jpool = ctx.enter_context(tc.tile_pool(name="junk", bufs=3))
    tpool = ctx.enter_context(tc.tile_pool(name="tree", bufs=1))

    def make_tree_ops(b0, b1, jp):
        # fold the 2x3 copied quarters of blocks (b0, b1): [128,2,3072]
        # -> treeb cols b0, b1.  Returns closures to dribble into the DVE
        # stream a few per block.
        v = jp.rearrange("p (b m) -> p b m", b=2)
        tX = tpool.tile([P, 2 * 1024], fp16, tag="tX", name=f"tX{b1}")
        tY = tpool.tile([P, 2 * 1024], fp16, tag="tY", name=f"tY{b1}")
        tC = tpool.tile([P, 2 * 512], fp16, tag="tC", name=f"tC{b1}")
        tD = tpool.tile([P, 2 * 256], fp16, tag="tD", name=f"tD{b1}")
        tE = tpool.tile([P, 2 * 128], fp16, tag="tE", name=f"tE{b1}")
        tXv = tX.rearrange("p (b m) -> p b m", b=2)
        tYv = tY.rearrange("p (b m) -> p b m", b=2)
        tCv = tC.rearrange("p (b m) -> p b m", b=2)
        tDv = tD.rearrange("p (b m) -> p b m", b=2)
        tEv = tE.rearrange("p (b m) -> p b m", b=2)
        return [
            lambda: vec.tensor_tensor(out=tXv, in0=v[:, :, 0:1024],
                                      in1=v[:, :, 1024:2048], op=Alu.max),
            lambda: vec.tensor_tensor(out=tYv, in0=tXv,
                                      in1=v[:, :, 2048:3072], op=Alu.max),
            lambda: vec.tensor_tensor(out=tCv, in0=tYv[:, :, 0:512],
                                      in1=tYv[:, :, 512:1024], op=Alu.max),
            lambda: vec.tensor_tensor(out=tDv, in0=tCv[:, :, 0:256],
                                      in1=tCv[:, :, 256:512], op=Alu.max),
            lambda: vec.tensor_tensor(out=tEv, in0=tDv[:, :, 0:128],
                                      in1=tDv[:, :, 128:256], op=Alu.max),
            lambda: vec.tensor_reduce(treeb[:, b0:b0 + 1], tEv[:, 0, :],
                                      axis=X, op=Alu.max),
            lambda: vec.tensor_reduce(treeb[:, b1:b1 + 1], tEv[:, 1, :],
                                      axis=X, op=Alu.max),
        ]

    # per-chunk tail state: as soon as the trees for an 8-column chunk of
    # blocks have been emitted, fold that chunk through negate/clamp/sqrt/
    # pixel-loss so only the last chunk remains after the final block.
    md = t("md", [P, NB])
    dist = t("dist", [P, NB])
    pix = t("pix", [P, NB])
    tail_done = [0]     # columns fully pushed through the chunk tail
    cols_ready = [0]    # columns whose tree ops are fully emitted

    def emit_chunk_tails():
        while tail_done[0] + 8 <= cols_ready[0]:
            c0, c1 = tail_done[0], tail_done[0] + 8
            tail_done[0] = c1
            vec.tensor_tensor(out=md[:, c0:c1], in0=dirb[:, c0:c1],
                              in1=treeb[:, c0:c1], op=Alu.max)
            vec.tensor_scalar_mul(md[:, c0:c1], md[:, c0:c1], -1.0)
            vec.tensor_scalar_max(md[:, c0:c1], md[:, c0:c1], 1e-12)
            nc.scalar.sqrt(dist[:, c0:c1], md[:, c0:c1])
            vec.tensor_tensor(out=pix[:, c0:c1], in0=dist[:, c0:c1],
                              in1=cc[:, c0:c1], op=Alu.mult)
            vec.scalar_tensor_tensor(out=pix[:, c0:c1], in0=lnc[:, c0:c1],
                                     scalar=-W_RATE, in1=pix[:, c0:c1],
                                     op0=Alu.mult, op1=Alu.add)

    tree_q = []
    jpair = None
    for beta in range(NB):
        tiles = [pp_psum.tile([P, CH], f32, tag="ps", name=f"ps{beta}_{q}")
                 for q in range(4)]
        for q in range(4):
            for j in range(2):
                nc.tensor.matmul(
                    tiles[q][:, j * 512:(j + 1) * 512],
                    lhsT[0:K_DIM, beta * P:(beta + 1) * P],
                    rhs[0:K_DIM,
                        q * CH + j * 512: q * CH + (j + 1) * 512],
                    start=True, stop=True,
                )
        # DVE reduces quarter 0 straight off PSUM (frees the tile early)
        vec.tensor_reduce(dirb[:, beta:beta + 1], tiles[0], axis=X,
                          op=Alu.max)
        # ScalarE evacuates quarters 1-3 as fp16 into the pair buffer
        half = beta % 2
        if half == 0:
            jpair = jpool.tile([P, 2 * 3072], fp16, tag="jp",
                               name=f"jp{beta}")
        for q in (1, 2, 3):
            nc.scalar.copy(
                out=jpair[:, half * 3072 + (q - 1) * CH:
                          half * 3072 + q * CH],
                in_=tiles[q])
        # dribble pending fold-tree ops behind this block's PSUM work
        # (flush everything near the end so the tail chain starts early)
        for _ in range(5 if beta < 28 else 16):
            if tree_q:
                tree_q.pop(0)()
                if not tree_q:
                    cols_ready[0] += 2
        emit_chunk_tails()
        if half == 1:
            tree_q += make_tree_ops(beta - 1, beta, jpair)
    while tree_q:
        tree_q.pop(0)()
        if not tree_q:
            cols_ready[0] += 2
    emit_chunk_tails()
    assert tail_done[0] == NB, tail_done

    # ---------------- tail: final row sums ----------
    sums = t("sums", [P, 1])
    vec.reduce_sum(sums, pix, axis=X)
    dma(out=out_ap, in_=sums)


def _build():
    from contextlib import ExitStack

    import concourse.bacc as bacc
    import concourse.tile as tile
    from concourse import mybir

    f32 = mybir.dt.float32
    nc = bacc.Bacc("TRN2", debug=False, enable_asserts=False, num_devices=B)
    ins = {
        "pred_quat": nc.dram_tensor("pred_quat", [N, 4], f32,
                                    kind="ExternalInput").ap(),
        "pred_trans": nc.dram_tensor("pred_trans", [N, 3], f32,
                                     kind="ExternalInput").ap(),
        "pred_conf": nc.dram_tensor("pred_conf", [N, 1], f32,
                                    kind="ExternalInput").ap(),
        "pose": nc.dram_tensor("pose", [3, 4], f32, kind="ExternalInput").ap(),
        "points": nc.dram_tensor("points", [N, 3], f32,
                                 kind="ExternalInput").ap(),
    }
    out_ap = nc.dram_tensor("out_sums", [P, 1], f32, kind="ExternalOutput").ap()
    with tile.TileContext(nc) as tc:
        with ExitStack() as ctx:
            _emit(ctx, tc, out_ap, ins)
    nc.compile()
    return nc


def _get_nc():
    if "nc" not in _cache:
        _cache["nc"] = _build()
    return _cache["nc"]


def _numpy_reference(pred_quat, pred_trans, pred_conf, pose, points, cls_id):
    """Full-precision numpy fallback (used only for the non-symmetric branch)."""
    q = pred_quat.astype(np.float64)
    q = q / np.clip(np.linalg.norm(q, axis=-1, keepdims=True), 1e-8, None)
    w, x, y, z = q[..., 0], q[..., 1], q[..., 2], q[..., 3]
    r = np.stack([
        1 - 2 * (y * y + z * z), 2 * (x * y - w * z), 2 * (x * z + w * y),
        2 * (x * y + w * z), 1 - 2 * (x * x + z * z), 2 * (y * z - w * x),
        2 * (x * z - w * y), 2 * (y * z + w * x), 1 - 2 * (x * x + y * y),
    ], axis=-1).reshape(q.shape[:-1] + (3, 3))
    gt_r = pose[:, :3, :3].astype(np.float64)
    gt_t = pose[:, :3, 3].astype(np.float64)
    pc = points.astype(np.float64) - gt_t[:, None, :]
    pm = np.einsum("bkj,bnj->bnk", gt_r, pc)
    ppred = np.einsum("bnij,bnj->bni", r, pm) + pred_trans.astype(np.float64)
    tgt = points.astype(np.float64)
    if int(cls_id[0]) in SYM_CLASS_IDS:
        aa = np.sum(ppred * ppred, axis=-1)
        bb2 = np.sum(tgt * tgt, axis=-1)
        ab = np.einsum("bnd,bmd->bnm", ppred, tgt)
        d2 = aa[:, :, None] + bb2[:, None, :] - 2.0 * ab
        loss_dist = np.sqrt(np.maximum(d2, 1e-12)).min(axis=2)
    else:
        loss_dist = np.linalg.norm(ppred - tgt, axis=2)
    c = np.clip(pred_conf[..., 0].astype(np.float64), 1e-4, 1.0)
    return np.float32(np.mean(loss_dist * c - W_RATE * np.log(c)))


def kernel(pred_quat, pred_trans, pred_conf, pose, points, cls_id):
    pred_quat = _np_f32(pred_quat)
    pred_trans = _np_f32(pred_trans)
    pred_conf = _np_f32(pred_conf)
    pose = _np_f32(pose)
    points = _np_f32(points)
    cls_id = np.asarray(cls_id)

    assert pred_quat.shape == (B, N, 4), pred_quat.shape

    if int(cls_id[0]) not in SYM_CLASS_IDS:
        return np.array(
            _numpy_reference(pred_quat, pred_trans, pred_conf, pose, points,
                             cls_id),
            dtype=np.float32)

    from concourse.bass_utils import run_bass_kernel_spmd

    nc = _get_nc()
    in_maps = [
        {
            "pred_quat": np.ascontiguousarray(pred_quat[c]),
            "pred_trans": np.ascontiguousarray(pred_trans[c]),
            "pred_conf": np.ascontiguousarray(pred_conf[c]),
            "pose": np.ascontiguousarray(pose[c]),
            "points": np.ascontiguousarray(points[c]),
        }
        for c in range(B)
    ]
    res = run_bass_kernel_spmd(nc, in_maps, core_ids=list(range(B)))
    total = np.float64(0.0)
    for r in res.results:
        total += np.sum(r["out_sums"].astype(np.float64))
    return np.array(total / (B * N), dtype=np.float32)
